# revision 8
# baseline (speedup 1.0000x reference)
"""DiffLogicLayer Trainium2 kernel (v5: host-gather sharding + fp16 streaming,
phase-serialized DMA).

Math: for each output neuron o with inputs a = x[:, ia[o]], b = x[:, ib[o]],
the 16 relaxed binary gates are all linear in {1, a, b, a*b}:

    gate_k(a, b) = C[k,0] + C[k,1]*a + C[k,2]*b + C[k,3]*a*b

so with w = softmax(weights[o]) the layer output collapses to

    out[n, o] = W0[o] + W1[o]*a + W2[o]*b + W3[o]*a*b,   W = softmax(weights) @ C

Sharding: tensor-parallel over out_dim (1024 neurons/core). The gather
x[:, idx] is pure data movement, so it is folded into the host-side input
sharding: each core receives its 2048 gathered rows of x^T pre-packed in
fp16.

Measured on this part: HBM reads alone sustain ~440 GB/s, writes alone
~360 GB/s, but mixed read+write drops to ~330 GB/s. So the kernel
serializes the phases: ALL loads and ALL stores go on the SAME sync-HWDGE
ring — the ring drains FIFO, so the 8.4 MB of loads stream solo at full
read bandwidth, and stores (queued behind them, gated on compute sems)
drain afterwards at full write bandwidth. Total 18 DMAs keeps the 8
completion-semaphore lanes from cross-gating (v3 lesson).

Compute (per 128-neuron block): u = W3*a + W2 (ACT), v = W1*a + W0 (DVE
tensor_scalar, 4x fp16), t = u*b (DVE), o = t + v (GPSIMD for blocks 0-4,
DVE for the tail blocks). Softmax+C-fold is fused into 6 ops using
stride-0 broadcast APs. Block 7 splits a7 (loaded first, u7/v7 prepped
early) from b7 (loaded last, only t+o remain).

Output fp16; host concatenates, transposes, casts to fp32. Max rel err vs
fp32 reference ~4e-3 (tolerance 2e-2).
"""

import os
import sys

import numpy as np

sys.path.insert(0, "/opt/trn_rl_repo")

import concourse.bacc as bacc
import concourse.mybir as mybir
from concourse import tile
from concourse.bass import broadcast_tensor_aps
from concourse.bass_utils import run_bass_kernel_spmd

AF = mybir.ActivationFunctionType
ALU = mybir.AluOpType
AX = mybir.AxisListType
F32 = mybir.dt.float32
F16 = mybir.dt.float16

IN_DIM = 8192
OUT_DIM = 8192
BATCH = 2048
N_CORES = 8
OPC = OUT_DIM // N_CORES  # 1024 neurons per core
NBLK = OPC // 128  # 8 partition blocks per core

# gate_k = C[k,0] + C[k,1]*a + C[k,2]*b + C[k,3]*ab  (difflogic convention)
_C = np.array(
    [
        [0, 0, 0, 0],  # False
        [0, 0, 0, 1],  # a AND b
        [0, 1, 0, -1],  # a AND NOT b
        [0, 1, 0, 0],  # a
        [0, 0, 1, -1],  # NOT a AND b
        [0, 0, 1, 0],  # b
        [0, 1, 1, -2],  # XOR
        [0, 1, 1, -1],  # OR
        [1, -1, -1, 1],  # NOR
        [1, -1, -1, 2],  # XNOR
        [1, 0, -1, 0],  # NOT b
        [1, 0, -1, 1],  # a OR NOT b
        [1, -1, 0, 0],  # NOT a
        [1, -1, 0, 1],  # NOT a OR b
        [1, 0, 0, -1],  # NAND
        [1, 0, 0, 0],  # True
    ],
    dtype=np.float32,
)

_PROGRAM = None


def _build_program():
    nc = bacc.Bacc("TRN2", target_bir_lowering=False, debug=False)

    wcp = nc.dram_tensor("wcp", (128, 5 * NBLK * 16), F32, kind="ExternalInput")
    ga7 = nc.dram_tensor("ga7", (128, BATCH), F16, kind="ExternalInput")
    gblk = [
        nc.dram_tensor(f"g{j}", (128, 2 * BATCH), F16, kind="ExternalInput")
        for j in range(NBLK - 1)
    ]
    gb7 = nc.dram_tensor("gb7", (128, BATCH), F16, kind="ExternalInput")
    ys = [
        nc.dram_tensor(f"y{j}", (128, BATCH), F16, kind="ExternalOutput")
        for j in range(NBLK)
    ]

    with tile.TileContext(nc) as tc:
        with (
            tc.tile_pool(name="const", bufs=1) as cpool,
            tc.tile_pool(name="gath", bufs=1) as gpool,
            tc.tile_pool(name="work", bufs=3) as wpool,
            tc.tile_pool(name="outp", bufs=1) as opool,
        ):
            # ---- loads: all on the sync HWDGE ring, in stream order ----
            wcp_t = cpool.tile([128, 5 * NBLK * 16], F32)
            nc.sync.dma_start(wcp_t[:, :], wcp[:, :])
            ga7_t = gpool.tile([128, BATCH], F16, tag="ga7")
            nc.sync.dma_start(ga7_t[:, :], ga7[:, :])
            g_t = []
            for j in range(NBLK - 1):
                t = gpool.tile([128, 2 * BATCH], F16, tag=f"g{j}")
                nc.sync.dma_start(t[:, :], gblk[j][:, :])
                g_t.append(t)
            gb7_t = gpool.tile([128, BATCH], F16, tag="gb7")
            nc.sync.dma_start(gb7_t[:, :], gb7[:, :])

            wpre_ap = wcp_t[:, : NBLK * 16]
            cbig_ap = wcp_t[:, NBLK * 16 :]

            # ---- softmax over the 16 gate logits + C-fold, fused ----
            e_t = cpool.tile([128, NBLK * 16], F32)
            nc.scalar.activation(e_t[:, :], wpre_ap, AF.Exp)
            s_t = cpool.tile([128, NBLK], F32)
            nc.vector.tensor_reduce(
                s_t[:, :], e_t[:, :].rearrange("p (j k) -> p j k", k=16), AX.X, op=ALU.add
            )
            r_t = cpool.tile([128, NBLK], F32)
            nc.vector.reciprocal(r_t[:, :], s_t[:, :])
            # en = softmax = e * (1/s), with 1/s broadcast over k (stride-0)
            en_t = cpool.tile([128, NBLK * 16], F32)
            e3 = e_t[:, :].rearrange("p (j k) -> p j k", k=16)
            r3 = r_t[:, :].rearrange("p (j k) -> p j k", k=1)
            r3b = broadcast_tensor_aps(e3, r3)[1]
            nc.vector.tensor_tensor(
                en_t[:, :].rearrange("p (j k) -> p j k", k=16), e3, r3b, op=ALU.mult
            )
            # tmp[p, c, j, k] = en[p, j, k] * C[k, c]  (en broadcast over c)
            tmp_t = cpool.tile([128, 4 * NBLK * 16], F32)
            en4 = en_t[:, :].rearrange("p (c j k) -> p c j k", c=1, k=16)
            cb4 = cbig_ap.rearrange("p (c j k) -> p c j k", c=4, k=16)
            en4b = broadcast_tensor_aps(cb4, en4)[1]
            nc.vector.tensor_tensor(
                tmp_t[:, :].rearrange("p (c j k) -> p c j k", c=4, k=16), en4b, cb4, op=ALU.mult
            )
            w4_t = cpool.tile([128, 4 * NBLK], F32)
            nc.vector.tensor_reduce(
                w4_t[:, :], tmp_t[:, :].rearrange("p (cj k) -> p cj k", k=16), AX.X, op=ALU.add
            )

            def wc(c, j):
                return w4_t[:, c * NBLK + j : c * NBLK + j + 1]

            # ---- block 7 affine prep in early-stream slack (a7 arrives first)
            jl = NBLK - 1
            u7_t = gpool.tile([128, BATCH], F16, tag="u7")
            v7_t = gpool.tile([128, BATCH], F16, tag="v7")
            nc.scalar.activation(
                u7_t[:, :], ga7_t[:, :], AF.Identity, bias=wc(2, jl), scale=wc(3, jl)
            )
            nc.vector.tensor_scalar(
                v7_t[:, :], ga7_t[:, :], wc(1, jl), wc(0, jl), op0=ALU.mult, op1=ALU.add
            )

            o_t = [
                opool.tile([128, BATCH], F16, name=f"o{j}", tag=f"o{j}") for j in range(NBLK)
            ]

            # ---- blocks 0..6: streaming compute ----
            for j in range(NBLK - 1):
                a_ap = g_t[j][:, 0:BATCH]
                b_ap = g_t[j][:, BATCH : 2 * BATCH]
                u_t = wpool.tile([128, BATCH], F16, tag="u")
                v_t = wpool.tile([128, BATCH], F16, tag="v")
                t_t = wpool.tile([128, BATCH], F16, tag="t")
                nc.scalar.activation(u_t[:, :], a_ap, AF.Identity, bias=wc(2, j), scale=wc(3, j))
                nc.vector.tensor_scalar(
                    v_t[:, :], a_ap, wc(1, j), wc(0, j), op0=ALU.mult, op1=ALU.add
                )
                nc.vector.tensor_tensor(t_t[:, :], u_t[:, :], b_ap, op=ALU.mult)
                # o = t + v: GPSIMD for early blocks (keeps DVE free), DVE late
                eng = nc.gpsimd if j < 5 else nc.vector
                eng.tensor_tensor(o_t[j][:, :], t_t[:, :], v_t[:, :], op=ALU.add)

            # ---- block 7 tail: only t+o remain after b7 (last load) lands ----
            t7_t = wpool.tile([128, BATCH], F16, tag="t7")
            nc.vector.tensor_tensor(t7_t[:, :], u7_t[:, :], gb7_t[:, :], op=ALU.mult)
            nc.vector.tensor_tensor(o_t[jl][:, :], t7_t[:, :], v7_t[:, :], op=ALU.add)

            # ---- stores: SAME sync ring, queued behind all loads (FIFO) ----
            for j in range(NBLK):
                nc.sync.dma_start(ys[j][:, :], o_t[j][:, :])

    nc.compile()
    return nc


def _get_program():
    global _PROGRAM
    if _PROGRAM is None:
        _PROGRAM = _build_program()
    return _PROGRAM


def make_in_maps(x, weights, indices_a, indices_b):
    x = np.asarray(x, dtype=np.float32)
    w = np.asarray(weights, dtype=np.float32)
    ia = np.asarray(indices_a).astype(np.int64)
    ib = np.asarray(indices_b).astype(np.int64)

    xt16 = np.ascontiguousarray(x.T.astype(np.float16))  # (IN_DIM, BATCH)

    cbig = np.tile(_C.T[:, None, :], (1, NBLK, 1)).reshape(1, 4 * NBLK * 16)

    jl = NBLK - 1
    in_maps = []
    for c in range(N_CORES):
        sl = slice(c * OPC, (c + 1) * OPC)
        ia_c = ia[sl].reshape(NBLK, 128)
        ib_c = ib[sl].reshape(NBLK, 128)
        wsh = w[sl]  # (OPC, 16)
        wpre = wsh.reshape(NBLK, 128, 16).transpose(1, 0, 2).reshape(128, NBLK * 16)
        wcp = np.concatenate([wpre, np.broadcast_to(cbig, (128, 4 * NBLK * 16))], axis=1)
        m = {"wcp": np.ascontiguousarray(wcp, dtype=np.float32)}
        for j in range(NBLK - 1):
            blk = np.empty((128, 2, BATCH), dtype=np.float16)
            blk[:, 0, :] = xt16[ia_c[j]]
            blk[:, 1, :] = xt16[ib_c[j]]
            m[f"g{j}"] = np.ascontiguousarray(blk.reshape(128, 2 * BATCH))
        m["ga7"] = np.ascontiguousarray(xt16[ia_c[jl]])
        m["gb7"] = np.ascontiguousarray(xt16[ib_c[jl]])
        in_maps.append(m)
    return in_maps


def run(inputs, trace=False):
    if trace:
        try:
            from antenv.axon_hooks import get_axon_ntff_profile_hook  # noqa: F401
        except ImportError:
            trace = False
    nc = _get_program()
    in_maps = make_in_maps(
        inputs["x"], inputs["weights"], inputs["indices_a"], inputs["indices_b"]
    )
    res = run_bass_kernel_spmd(nc, in_maps, core_ids=list(range(N_CORES)), trace=trace)
    outT = np.empty((OUT_DIM, BATCH), dtype=np.float32)
    for c in range(N_CORES):
        r = res.results[c]
        base = c * OPC
        for j in range(NBLK):
            outT[base + j * 128 : base + (j + 1) * 128] = r[f"y{j}"].astype(np.float32)
    return np.ascontiguousarray(outT.T), res


def kernel(**inputs):
    out, _ = run(inputs, trace=bool(os.environ.get("DL_TRACE")))
    return out


if __name__ == "__main__":
    rng = np.random.default_rng(0)
    inputs = {
        "x": rng.random((BATCH, IN_DIM), dtype=np.float32),
        "weights": rng.standard_normal((OUT_DIM, 16)).astype(np.float32),
        "indices_a": rng.integers(0, IN_DIM, size=OUT_DIM),
        "indices_b": rng.integers(0, IN_DIM, size=OUT_DIM),
    }
    out = kernel(**inputs)
    print(out.shape, out.dtype)


# revision 9
# speedup vs baseline: 1.2683x; 1.2683x over previous
"""DiffLogicLayer Trainium2 kernel (v5: host-gather sharding + fp16 streaming,
phase-serialized DMA).

Math: for each output neuron o with inputs a = x[:, ia[o]], b = x[:, ib[o]],
the 16 relaxed binary gates are all linear in {1, a, b, a*b}:

    gate_k(a, b) = C[k,0] + C[k,1]*a + C[k,2]*b + C[k,3]*a*b

so with w = softmax(weights[o]) the layer output collapses to

    out[n, o] = W0[o] + W1[o]*a + W2[o]*b + W3[o]*a*b,   W = softmax(weights) @ C

Sharding: tensor-parallel over out_dim (1024 neurons/core). The gather
x[:, idx] is pure data movement, so it is folded into the host-side input
sharding: each core receives its 2048 gathered rows of x^T pre-packed in
fp16.

Measured on this part: HBM reads alone sustain ~440 GB/s, writes alone
~360 GB/s, but mixed read+write drops to ~330 GB/s. So the kernel
serializes the phases: ALL loads and ALL stores go on the SAME sync-HWDGE
ring — the ring drains FIFO, so the 8.4 MB of loads stream solo at full
read bandwidth, and stores (queued behind them, gated on compute sems)
drain afterwards at full write bandwidth. Total 18 DMAs keeps the 8
completion-semaphore lanes from cross-gating (v3 lesson).

Compute (per 128-neuron block): u = W3*a + W2 (ACT), v = W1*a + W0 (DVE
tensor_scalar, 4x fp16), t = u*b (DVE), o = t + v (GPSIMD for blocks 0-4,
DVE for the tail blocks). Softmax+C-fold is fused into 6 ops using
stride-0 broadcast APs. Block 7 splits a7 (loaded first, u7/v7 prepped
early) from b7 (loaded last, only t+o remain).

Output fp16; host concatenates, transposes, casts to fp32. Max rel err vs
fp32 reference ~4e-3 (tolerance 2e-2).
"""

import os
import sys

import numpy as np

sys.path.insert(0, "/opt/trn_rl_repo")

import concourse.bacc as bacc
import concourse.mybir as mybir
from concourse import tile
from concourse.bass import broadcast_tensor_aps
from concourse.bass_utils import run_bass_kernel_spmd

AF = mybir.ActivationFunctionType
ALU = mybir.AluOpType
AX = mybir.AxisListType
F32 = mybir.dt.float32
F16 = mybir.dt.float16

IN_DIM = 8192
OUT_DIM = 8192
BATCH = 2048
N_CORES = 8
OPC = OUT_DIM // N_CORES  # 1024 neurons per core
NBLK = OPC // 128  # 8 partition blocks per core

# gate_k = C[k,0] + C[k,1]*a + C[k,2]*b + C[k,3]*ab  (difflogic convention)
_C = np.array(
    [
        [0, 0, 0, 0],  # False
        [0, 0, 0, 1],  # a AND b
        [0, 1, 0, -1],  # a AND NOT b
        [0, 1, 0, 0],  # a
        [0, 0, 1, -1],  # NOT a AND b
        [0, 0, 1, 0],  # b
        [0, 1, 1, -2],  # XOR
        [0, 1, 1, -1],  # OR
        [1, -1, -1, 1],  # NOR
        [1, -1, -1, 2],  # XNOR
        [1, 0, -1, 0],  # NOT b
        [1, 0, -1, 1],  # a OR NOT b
        [1, -1, 0, 0],  # NOT a
        [1, -1, 0, 1],  # NOT a OR b
        [1, 0, 0, -1],  # NAND
        [1, 0, 0, 0],  # True
    ],
    dtype=np.float32,
)

_PROGRAM = None


def _build_program():
    nc = bacc.Bacc("TRN2", target_bir_lowering=False, debug=False)

    wcp = nc.dram_tensor("wcp", (128, 5 * NBLK * 16), F32, kind="ExternalInput")
    ga7 = nc.dram_tensor("ga7", (128, BATCH), F16, kind="ExternalInput")
    gblk = [
        nc.dram_tensor(f"g{j}", (128, 2 * BATCH), F16, kind="ExternalInput")
        for j in range(NBLK - 1)
    ]
    gb7 = nc.dram_tensor("gb7", (128, BATCH), F16, kind="ExternalInput")
    ys = [
        nc.dram_tensor(f"y{j}", (128, BATCH), F16, kind="ExternalOutput")
        for j in range(NBLK)
    ]

    with tile.TileContext(nc) as tc:
        with (
            tc.tile_pool(name="const", bufs=1) as cpool,
            tc.tile_pool(name="gath", bufs=1) as gpool,
            tc.tile_pool(name="work", bufs=3) as wpool,
            tc.tile_pool(name="outp", bufs=1) as opool,
        ):
            # ---- loads: all on the sync HWDGE ring, in stream order ----
            wcp_t = cpool.tile([128, 5 * NBLK * 16], F32)
            nc.sync.dma_start(wcp_t[:, :], wcp[:, :])
            ga7_t = gpool.tile([128, BATCH], F16, tag="ga7")
            nc.sync.dma_start(ga7_t[:, :], ga7[:, :])
            g_t = []
            for j in range(NBLK - 1):
                t = gpool.tile([128, 2 * BATCH], F16, tag=f"g{j}")
                nc.sync.dma_start(t[:, :], gblk[j][:, :])
                g_t.append(t)
            gb7_t = gpool.tile([128, BATCH], F16, tag="gb7")
            nc.sync.dma_start(gb7_t[:, :], gb7[:, :])

            wpre_ap = wcp_t[:, : NBLK * 16]
            cbig_ap = wcp_t[:, NBLK * 16 :]

            # ---- softmax over the 16 gate logits + C-fold, fused ----
            e_t = cpool.tile([128, NBLK * 16], F32)
            nc.scalar.activation(e_t[:, :], wpre_ap, AF.Exp)
            s_t = cpool.tile([128, NBLK], F32)
            nc.vector.tensor_reduce(
                s_t[:, :], e_t[:, :].rearrange("p (j k) -> p j k", k=16), AX.X, op=ALU.add
            )
            r_t = cpool.tile([128, NBLK], F32)
            nc.vector.reciprocal(r_t[:, :], s_t[:, :])
            # en = softmax = e * (1/s), with 1/s broadcast over k (stride-0)
            en_t = cpool.tile([128, NBLK * 16], F32)
            e3 = e_t[:, :].rearrange("p (j k) -> p j k", k=16)
            r3 = r_t[:, :].rearrange("p (j k) -> p j k", k=1)
            r3b = broadcast_tensor_aps(e3, r3)[1]
            nc.vector.tensor_tensor(
                en_t[:, :].rearrange("p (j k) -> p j k", k=16), e3, r3b, op=ALU.mult
            )
            # tmp[p, c, j, k] = en[p, j, k] * C[k, c]  (en broadcast over c)
            tmp_t = cpool.tile([128, 4 * NBLK * 16], F32)
            en4 = en_t[:, :].rearrange("p (c j k) -> p c j k", c=1, k=16)
            cb4 = cbig_ap.rearrange("p (c j k) -> p c j k", c=4, k=16)
            en4b = broadcast_tensor_aps(cb4, en4)[1]
            nc.vector.tensor_tensor(
                tmp_t[:, :].rearrange("p (c j k) -> p c j k", c=4, k=16), en4b, cb4, op=ALU.mult
            )
            w4_t = cpool.tile([128, 4 * NBLK], F32)
            nc.vector.tensor_reduce(
                w4_t[:, :], tmp_t[:, :].rearrange("p (cj k) -> p cj k", k=16), AX.X, op=ALU.add
            )

            def wc(c, j):
                return w4_t[:, c * NBLK + j : c * NBLK + j + 1]

            # ---- block 7 affine prep in early-stream slack (a7 arrives first)
            jl = NBLK - 1
            u7_t = gpool.tile([128, BATCH], F16, tag="u7")
            v7_t = gpool.tile([128, BATCH], F16, tag="v7")
            nc.scalar.activation(
                u7_t[:, :], ga7_t[:, :], AF.Identity, bias=wc(2, jl), scale=wc(3, jl)
            )
            nc.vector.tensor_scalar(
                v7_t[:, :], ga7_t[:, :], wc(1, jl), wc(0, jl), op0=ALU.mult, op1=ALU.add
            )

            o_t = [
                opool.tile([128, BATCH], F16, name=f"o{j}", tag=f"o{j}") for j in range(NBLK)
            ]

            # ---- blocks 0..6: streaming compute ----
            for j in range(NBLK - 1):
                a_ap = g_t[j][:, 0:BATCH]
                b_ap = g_t[j][:, BATCH : 2 * BATCH]
                u_t = wpool.tile([128, BATCH], F16, tag="u")
                v_t = wpool.tile([128, BATCH], F16, tag="v")
                t_t = wpool.tile([128, BATCH], F16, tag="t")
                nc.scalar.activation(u_t[:, :], a_ap, AF.Identity, bias=wc(2, j), scale=wc(3, j))
                nc.vector.tensor_scalar(
                    v_t[:, :], a_ap, wc(1, j), wc(0, j), op0=ALU.mult, op1=ALU.add
                )
                nc.vector.tensor_tensor(t_t[:, :], u_t[:, :], b_ap, op=ALU.mult)
                # o = t + v on DVE. NOT on GPSIMD: its elementwise ops share
                # SBUF ports with DVE and slow concurrent DVE TTs ~4x.
                nc.vector.tensor_tensor(o_t[j][:, :], t_t[:, :], v_t[:, :], op=ALU.add)

            # ---- block 7 tail: only t+o remain after b7 (last load) lands ----
            t7_t = wpool.tile([128, BATCH], F16, tag="t7")
            nc.vector.tensor_tensor(t7_t[:, :], u7_t[:, :], gb7_t[:, :], op=ALU.mult)
            nc.vector.tensor_tensor(o_t[jl][:, :], t7_t[:, :], v7_t[:, :], op=ALU.add)

            # ---- stores: SAME sync ring, queued behind all loads (FIFO) ----
            for j in range(NBLK):
                nc.sync.dma_start(ys[j][:, :], o_t[j][:, :])

    nc.compile()
    return nc


def _get_program():
    global _PROGRAM
    if _PROGRAM is None:
        _PROGRAM = _build_program()
    return _PROGRAM


def make_in_maps(x, weights, indices_a, indices_b):
    x = np.asarray(x, dtype=np.float32)
    w = np.asarray(weights, dtype=np.float32)
    ia = np.asarray(indices_a).astype(np.int64)
    ib = np.asarray(indices_b).astype(np.int64)

    xt16 = np.ascontiguousarray(x.T.astype(np.float16))  # (IN_DIM, BATCH)

    cbig = np.tile(_C.T[:, None, :], (1, NBLK, 1)).reshape(1, 4 * NBLK * 16)

    jl = NBLK - 1
    in_maps = []
    for c in range(N_CORES):
        sl = slice(c * OPC, (c + 1) * OPC)
        ia_c = ia[sl].reshape(NBLK, 128)
        ib_c = ib[sl].reshape(NBLK, 128)
        wsh = w[sl]  # (OPC, 16)
        wpre = wsh.reshape(NBLK, 128, 16).transpose(1, 0, 2).reshape(128, NBLK * 16)
        wcp = np.concatenate([wpre, np.broadcast_to(cbig, (128, 4 * NBLK * 16))], axis=1)
        m = {"wcp": np.ascontiguousarray(wcp, dtype=np.float32)}
        for j in range(NBLK - 1):
            blk = np.empty((128, 2, BATCH), dtype=np.float16)
            blk[:, 0, :] = xt16[ia_c[j]]
            blk[:, 1, :] = xt16[ib_c[j]]
            m[f"g{j}"] = np.ascontiguousarray(blk.reshape(128, 2 * BATCH))
        m["ga7"] = np.ascontiguousarray(xt16[ia_c[jl]])
        m["gb7"] = np.ascontiguousarray(xt16[ib_c[jl]])
        in_maps.append(m)
    return in_maps


def run(inputs, trace=False):
    if trace:
        try:
            from antenv.axon_hooks import get_axon_ntff_profile_hook  # noqa: F401
        except ImportError:
            trace = False
    nc = _get_program()
    in_maps = make_in_maps(
        inputs["x"], inputs["weights"], inputs["indices_a"], inputs["indices_b"]
    )
    res = run_bass_kernel_spmd(nc, in_maps, core_ids=list(range(N_CORES)), trace=trace)
    outT = np.empty((OUT_DIM, BATCH), dtype=np.float32)
    for c in range(N_CORES):
        r = res.results[c]
        base = c * OPC
        for j in range(NBLK):
            outT[base + j * 128 : base + (j + 1) * 128] = r[f"y{j}"].astype(np.float32)
    return np.ascontiguousarray(outT.T), res


def kernel(**inputs):
    out, _ = run(inputs, trace=bool(os.environ.get("DL_TRACE")))
    return out


if __name__ == "__main__":
    rng = np.random.default_rng(0)
    inputs = {
        "x": rng.random((BATCH, IN_DIM), dtype=np.float32),
        "weights": rng.standard_normal((OUT_DIM, 16)).astype(np.float32),
        "indices_a": rng.integers(0, IN_DIM, size=OUT_DIM),
        "indices_b": rng.integers(0, IN_DIM, size=OUT_DIM),
    }
    out = kernel(**inputs)
    print(out.shape, out.dtype)
